# revision 16
# baseline (speedup 1.0000x reference)
"""Bass/Trainium2 kernel for nn_BailingMoELinearAttention.

Tensor-parallel over heads across 8 NeuronCores: each core owns 2 of the 16
heads (columns of Wqkv/Wg, rows of Wo). Per-core pipeline, fused per 128-token
chunk:
  qkvg projection (bf16 matmul) -> silu(q,k,v) / sigmoid(gate) -> per-head
  RMSNorm -> RoPE (host-precomputed cos/sin tables) -> chunked causal linear
  attention (running k^T v state, one PSUM bank per head) -> pre-gated
  activations g2 = o * g_norm_w * sigmoid(gate) and partial sum-of-squares.
Cross-core: AllReduce of per-token sum-of-squares (the group norm spans all 16
heads), per-token rstd scaling, then an AllToAll that exchanges the bf16 gated
activations so each core holds all 2048 inner columns for its 1024-token row
block, and a local out-projection against the full Wo. Host concatenates the 8
row blocks.
"""

import os
import sys

if "/opt/trn_rl_repo" not in sys.path:
    sys.path.insert(0, "/opt/trn_rl_repo")

import numpy as np
import ml_dtypes

import concourse.bass as bass
import concourse.tile as tile
from concourse import bacc, mybir
from concourse.bass_utils import run_bass_kernel_spmd
from concourse.masks import make_identity
from concourse.tile import add_dep_helper

BF16 = ml_dtypes.bfloat16

# Problem shape (hardcoded per contract).
T = 8192
HID = 2048
H = 16
D = 128
INNER = H * D
CHUNK = 128
NCHUNK = T // CHUNK  # 64
EPS = 1e-5
SCALE = D ** -0.5
ROPE_BASE = 600000.0
HALF = D // 2

N_CORES = 8
HPC = H // N_CORES          # 2 heads per core
CPC = HPC * D               # 256 inner cols per core
ROWS_PC = T // N_CORES      # 1024 output rows per core
RCHUNK = ROWS_PC // CHUNK   # 8 row-chunks per core in phase D

FP32 = mybir.dt.float32
BF = mybir.dt.bfloat16


def _build_program():
    nc = bacc.Bacc(
        "TRN2",
        target_bir_lowering=False,
        debug=False,
        num_devices=N_CORES,
    )

    # ---- DRAM I/O ----
    xt = nc.dram_tensor("xt", [NCHUNK, 128, HID // 128, CHUNK], BF,
                        kind="ExternalInput").ap()
    w = nc.dram_tensor("w", [128, HID // 128, 4 * CPC], BF,
                       kind="ExternalInput").ap()
    wo = nc.dram_tensor("wo", [128, HID // 128, HID], BF,
                        kind="ExternalInput").ap()
    cosd = nc.dram_tensor("cosd", [NCHUNK, CHUNK, D], BF,
                          kind="ExternalInput").ap()
    sind = nc.dram_tensor("sind", [NCHUNK, CHUNK, D], BF,
                          kind="ExternalInput").ap()
    qln = nc.dram_tensor("qln", [128, CPC], FP32, kind="ExternalInput").ap()
    kln = nc.dram_tensor("kln", [128, CPC], FP32, kind="ExternalInput").ap()
    gnw = nc.dram_tensor("gnw", [128, CPC], FP32, kind="ExternalInput").ap()
    maskt = nc.dram_tensor("maskt", [CHUNK, CHUNK], BF,
                           kind="ExternalInput").ap()
    out = nc.dram_tensor("out", [ROWS_PC, HID], FP32,
                         kind="ExternalOutput").ap()
    DEBUG = bool(os.environ.get("KB_DEBUG"))
    if DEBUG:
        dbg_o = nc.dram_tensor("dbg_o", [NCHUNK, 128, CPC], FP32,
                               kind="ExternalOutput").ap()
        dbg_ss = nc.dram_tensor("dbg_ss", [128, NCHUNK], FP32,
                                kind="ExternalOutput").ap()

    KO = HID // 128  # 16 k-chunks for the projections

    with tile.TileContext(nc) as tc:
        with (
            tc.tile_pool(name="const", bufs=1) as const,
            tc.tile_pool(name="persist", bufs=1) as persist,
            tc.tile_pool(name="dram", bufs=1, space="DRAM") as dram,
        ):
            # internal DRAM (A2A/RS split into 4 token-quarters so the
            # first three collectives overlap the chunk loop)
            TQ = T // 4
            a2a_in = [dram.tile([TQ, CPC], BF, name=f"a2a_in{q}")
                      for q in range(4)]
            a2a_out = [dram.tile([TQ, CPC], BF, name=f"a2a_out{q}")
                       for q in range(4)]
            ss_part = [dram.tile([TQ], FP32, name=f"ss_part{q}")
                       for q in range(4)]
            ss_mine = [dram.tile([TQ // N_CORES], FP32, name=f"ss_mine{q}")
                       for q in range(4)]

            # prefetch chunk 0 inputs ahead of the weight loads
            pre_xt = const.tile([128, KO, CHUNK], BF)
            nc.sync.dma_start(out=pre_xt, in_=xt[0])
            pre_cos = const.tile([CHUNK, D], BF)
            nc.sync.dma_start(out=pre_cos, in_=cosd[0])
            pre_sin = const.tile([CHUNK, D], BF)
            nc.sync.dma_start(out=pre_sin, in_=sind[0])

            # constants in SBUF
            w_s = const.tile([128, KO, 4 * CPC], BF)
            for ko in range(KO):
                nc.scalar.dma_start(out=w_s[:, ko, :], in_=w[:, ko, :])
            wo_s = const.tile([128, KO, HID], BF)
            qln_s = const.tile([128, CPC], FP32)
            nc.scalar.dma_start(out=qln_s, in_=qln)
            kln_s = const.tile([128, CPC], FP32)
            nc.scalar.dma_start(out=kln_s, in_=kln)
            gnw_s = const.tile([128, CPC], FP32)
            nc.scalar.dma_start(out=gnw_s, in_=gnw)
            maskt_s = const.tile([CHUNK, CHUNK], BF)
            nc.scalar.dma_start(out=maskt_s, in_=maskt)
            ident = const.tile([128, 128], BF)
            make_identity(nc, ident)
            identf = const.tile([128, 128], FP32)
            make_identity(nc, identf)
            eps_t = const.tile([128, 1], FP32)
            nc.vector.memset(eps_t, EPS)

            # accumulators living across the whole chunk loop
            ssm = persist.tile([128, RCHUNK], FP32)
            rmsm = persist.tile([128, RCHUNK], FP32)
            rstd_m = persist.tile([128, RCHUNK], FP32)
            ss_all = persist.tile([128, NCHUNK], FP32)     # partial sumsq
            s_sb = persist.tile([128, HPC, D], BF)         # state for o_inter
            nc.vector.memset(s_sb, 0.0)
            aiv = [a2a_in[q][:].rearrange("(n p) c -> n p c", p=CHUNK)
                   for q in range(4)]

            # ---- phase A/B/C: projections + attention, per 128-token chunk ----
            with (
                tc.tile_pool(name="xt_p", bufs=3) as xt_p,
                tc.tile_pool(name="trig", bufs=2) as trig,
                tc.tile_pool(name="work", bufs=2) as work,
                tc.tile_pool(name="small", bufs=4) as small,
                tc.tile_pool(name="pq", bufs=2, space="PSUM") as pq,
                tc.tile_pool(name="ptr", bufs=1, space="PSUM") as ptr,
                tc.tile_pool(name="po", bufs=1, space="PSUM") as po,
                tc.tile_pool(name="ps", bufs=1, space="PSUM") as ps,
            ):
                # one PSUM bank per head: a start=True from another group
                # sharing the bank would clear this group's has_written bits
                # and break the running accumulation
                s_psum = [ps.tile([128, D], FP32, tag=f"s{h}",
                                  name=f"s_psum{h}")
                          for h in range(HPC)]

                for n in range(NCHUNK):
                    # --- projection: qkvg chunk = x_chunk @ [Wq|Wk|Wv|Wg] ---
                    if n == 0:
                        xt_s = pre_xt
                    else:
                        xt_s = xt_p.tile([128, KO, CHUNK], BF)
                        nc.sync.dma_start(out=xt_s, in_=xt[n])
                    psum_qkvg = pq.tile([128, 4 * CPC], FP32)
                    for ko in range(KO):
                        for sl in range(2):
                            nc.tensor.matmul(
                                psum_qkvg[:, sl * 512:(sl + 1) * 512],
                                lhsT=xt_s[:, ko, :],
                                rhs=w_s[:, ko, sl * 512:(sl + 1) * 512],
                                start=(ko == 0),
                                stop=(ko == KO - 1),
                            )

                    # --- silu(qkv), sigmoid(gate) * g_norm_w ---
                    qkv_s = work.tile([128, 3 * CPC], FP32, tag="qkv")
                    nc.scalar.activation(
                        out=qkv_s, in_=psum_qkvg[:, :3 * CPC],
                        func=mybir.ActivationFunctionType.Silu,
                    )
                    sig_t = work.tile([128, CPC], BF, tag="sig")
                    nc.scalar.activation(
                        out=sig_t, in_=psum_qkvg[:, 3 * CPC:],
                        func=mybir.ActivationFunctionType.Sigmoid,
                    )
                    sg_t = work.tile([128, CPC], BF, tag="sg")
                    nc.vector.tensor_mul(sg_t, sig_t, gnw_s)

                    # --- per-head RMSNorm of q and k (over D=128) ---
                    sq_s = work.tile([128, 2 * CPC], FP32, tag="sq")
                    nc.vector.tensor_mul(sq_s, qkv_s[:, :2 * CPC],
                                         qkv_s[:, :2 * CPC])
                    ssum = small.tile([128, 4], FP32, tag="ssum")
                    nc.vector.reduce_sum(
                        out=ssum,
                        in_=sq_s.rearrange("p (g d) -> p g d", d=D),
                        axis=mybir.AxisListType.X,
                    )
                    rms = small.tile([128, 4], FP32, tag="rms")
                    nc.scalar.activation(
                        out=rms, in_=ssum,
                        func=mybir.ActivationFunctionType.Sqrt,
                        bias=eps_t, scale=1.0 / D,
                    )
                    rstd = small.tile([128, 4], FP32, tag="rstd")
                    nc.vector.reciprocal(out=rstd, in_=rms)
                    for g in range(4):
                        nc.vector.tensor_scalar_mul(
                            out=qkv_s[:, g * D:(g + 1) * D],
                            in0=qkv_s[:, g * D:(g + 1) * D],
                            scalar1=rstd[:, g:g + 1],
                        )
                    # ln weights (SCALE folded into qln on host)
                    nc.vector.tensor_mul(qkv_s[:, :CPC], qkv_s[:, :CPC], qln_s)
                    nc.vector.tensor_mul(qkv_s[:, CPC:2 * CPC],
                                         qkv_s[:, CPC:2 * CPC], kln_s)

                    # --- rope on q and k ---
                    if n == 0:
                        cos_t, sin_t = pre_cos, pre_sin
                    else:
                        cos_t = trig.tile([CHUNK, D], BF, tag="cos")
                        nc.sync.dma_start(out=cos_t, in_=cosd[n])
                        sin_t = trig.tile([CHUNK, D], BF, tag="sin")
                        nc.sync.dma_start(out=sin_t, in_=sind[n])
                    cos_v = cos_t.rearrange("p (h f) -> p h f", f=HALF)
                    sin_v = sin_t.rearrange("p (h f) -> p h f", f=HALF)

                    q_ro = work.tile([128, HPC, D], BF, tag="q_ro")
                    k_ro = work.tile([128, HPC, D], BF, tag="k_ro")
                    v_s = work.tile([128, HPC, D], BF, tag="v_s")
                    nc.gpsimd.tensor_copy(v_s.rearrange("p h d -> p (h d)"),
                                          qkv_s[:, 2 * CPC:])
                    for src_off, dst in ((0, q_ro), (CPC, k_ro)):
                        xv = qkv_s[:, src_off:src_off + CPC].rearrange(
                            "p (h d) -> p h d", d=D)
                        x1 = xv[:, :, :HALF]
                        x2 = xv[:, :, HALF:]
                        t1 = work.tile([128, HPC, HALF], FP32, tag="ropet1")
                        t2 = work.tile([128, HPC, HALF], FP32, tag="ropet2")
                        nc.vector.tensor_mul(t1, x1, cos_v)
                        nc.vector.tensor_mul(t2, x2, sin_v)
                        nc.vector.tensor_sub(dst[:, :, :HALF], t1, t2)
                        nc.vector.tensor_mul(t1, x2, cos_v)
                        nc.vector.tensor_mul(t2, x1, sin_v)
                        nc.vector.tensor_add(dst[:, :, HALF:], t1, t2)

                    # --- transposes: qT, kT (per head) via PE ---
                    qT = work.tile([128, HPC, D], BF, tag="qT")
                    kT = work.tile([128, HPC, D], BF, tag="kT")
                    for ro, tr in ((q_ro, qT), (k_ro, kT)):
                        for h in range(HPC):
                            tp = ptr.tile([128, 128], BF, tag="tr")
                            nc.tensor.transpose(tp, ro[:, h, :], ident)
                            nc.vector.tensor_copy(tr[:, h, :], tp)

                    # --- attention ---
                    psum_o = po.tile([128, CPC], FP32)
                    prev_last_mm = None
                    for h in range(HPC):
                        sc_p = ptr.tile([128, 128], FP32, tag="tr")
                        nc.tensor.matmul(sc_p, lhsT=kT[:, h, :],
                                         rhs=qT[:, h, :], start=True, stop=True)
                        sc_s = work.tile([CHUNK, CHUNK], BF, tag="sc")
                        nc.vector.tensor_mul(sc_s, sc_p, maskt_s)
                        mm_intra = nc.tensor.matmul(
                            psum_o[:, h * D:(h + 1) * D],
                            lhsT=sc_s, rhs=v_s[:, h, :],
                            start=True, stop=(n == 0))
                        if prev_last_mm is not None:
                            # psum_o's bank is shared by both heads' groups;
                            # this start=True clears the whole bank's
                            # has_written bits, so it must not land between
                            # the other head's accumulating matmuls
                            add_dep_helper(mm_intra.ins, prev_last_mm.ins,
                                           reason="shared o-bank group order")
                        last_mm = mm_intra
                        if n > 0:
                            last_mm = nc.tensor.matmul(
                                psum_o[:, h * D:(h + 1) * D],
                                lhsT=qT[:, h, :], rhs=s_sb[:, h, :],
                                start=False, stop=True)
                        prev_last_mm = last_mm
                        if n < NCHUNK - 1:
                            nc.tensor.matmul(s_psum[h],
                                             lhsT=k_ro[:, h, :],
                                             rhs=v_s[:, h, :],
                                             start=(n == 0),
                                             stop=(n == NCHUNK - 2))
                            nc.vector.tensor_copy(s_sb[:, h, :], s_psum[h])

                    # --- g2 = o * (gnw * sig) -> a2a_in (rstd applied later,
                    # it commutes with the out-projection row-wise) ---
                    g2_t = work.tile([128, CPC], BF, tag="g2")
                    nc.vector.tensor_mul(g2_t, psum_o, sg_t)
                    # store via gpsimd (SWDGE): keeps the sync HWDGE queue
                    # free of stores so xt prefetches are never head-of-line
                    # blocked behind not-yet-ready data
                    nc.scalar.dma_start(out=aiv[n // 16][n % 16], in_=g2_t)
                    # partial sumsq of o (f32, straight from psum)
                    osq = work.tile([128, CPC], FP32, tag="osq")
                    nc.scalar.activation(
                        out=osq, in_=psum_o,
                        func=mybir.ActivationFunctionType.Square,
                    )
                    nc.vector.reduce_sum(out=ss_all[:, n:n + 1], in_=osq,
                                         axis=mybir.AxisListType.X)
                    if DEBUG:
                        do_ = work.tile([128, CPC], FP32, tag="dbgo")
                        nc.vector.tensor_copy(do_, psum_o)
                        nc.scalar.dma_start(out=dbg_o[n], in_=do_)

                    if n % 16 == 15:
                        # quarter q complete: token-major ss slice, then
                        # A2A of the quarter's gated activations + RS of
                        # its sumsq -- the first three quarters' collectives
                        # overlap the remaining chunk loop
                        q = n // 16
                        ssT_p = ptr.tile([16, 128], FP32, tag="tr",
                                         name=f"ssT_p{q}")
                        nc.tensor.transpose(
                            ssT_p, ss_all[:, q * 16:(q + 1) * 16], identf)
                        ssT = small.tile([16, 128], FP32, tag="ssT",
                                         name=f"ssT{q}")
                        nc.vector.tensor_copy(ssT, ssT_p)
                        nc.scalar.dma_start(
                            out=ss_part[q][:].rearrange(
                                "(n c) -> n c", c=CHUNK),
                            in_=ssT)
                        nc.gpsimd.collective_compute(
                            "ReduceScatter",
                            mybir.AluOpType.add,
                            replica_groups=[list(range(N_CORES))],
                            ins=[ss_part[q][:].opt()],
                            outs=[ss_mine[q][:].opt()],
                        )
                        nc.gpsimd.collective_compute(
                            "AllToAll",
                            mybir.AluOpType.bypass,
                            replica_groups=[list(range(N_CORES))],
                            ins=[a2a_in[q][:].opt()],
                            outs=[a2a_out[q][:].opt()],
                        )
            if DEBUG:
                nc.scalar.dma_start(out=dbg_ss, in_=ss_all)


            def emit_rstd(q):
                nc.sync.dma_start(
                    out=ssm[:, q * 2:(q + 1) * 2],
                    in_=ss_mine[q][:].rearrange("(r c) -> c r", c=CHUNK))
                nc.scalar.activation(
                    out=rmsm[:, q * 2:(q + 1) * 2],
                    in_=ssm[:, q * 2:(q + 1) * 2],
                    func=mybir.ActivationFunctionType.Sqrt,
                    bias=eps_t, scale=1.0 / INNER,
                )
                nc.vector.reciprocal(
                    out=rstd_m[:, q * 2:(q + 1) * 2],
                    in_=rmsm[:, q * 2:(q + 1) * 2])

            for q in range(3):
                emit_rstd(q)
            nc.sync.dma_start(out=wo_s, in_=wo)

            # ---- phase D: local out-projection of my token rows ----
            # a2a_out[q][j, lt, f] = gated cols [j*CPC+f] of token
            # q*2048 + me*256 + lt;  rchunk r covers (q, lh) = (r//2, r%2)
            aov = [a2a_out[q][:].rearrange("(j t) c -> j t c", j=N_CORES)
                   for q in range(4)]
            with (
                tc.tile_pool(name="dwork", bufs=2) as dwork,
                tc.tile_pool(name="gl", bufs=16) as gl,
                tc.tile_pool(name="pout", bufs=2, space="PSUM") as pout,
            ):
                for r in range(RCHUNK):
                    q, lh = r // 2, r % 2
                    if r == 6:
                        emit_rstd(3)
                    out_psum = pout.tile([128, HID], FP32)
                    for ko in range(KO):
                        j, half = ko // 2, ko % 2
                        glt = gl.tile([128, CHUNK], BF, tag="glt")
                        nc.sync.dma_start(
                            out=glt,
                            in_=aov[q][j, lh * CHUNK:(lh + 1) * CHUNK,
                                       half * 128:(half + 1) * 128],
                            transpose=True,
                        )
                        for sl in range(HID // 512):
                            nc.tensor.matmul(
                                out_psum[:, sl * 512:(sl + 1) * 512],
                                lhsT=glt,
                                rhs=wo_s[:, ko, sl * 512:(sl + 1) * 512],
                                start=(ko == 0),
                                stop=(ko == KO - 1),
                            )
                    out_sb = dwork.tile([128, HID], FP32, tag="out_sb")
                    nc.vector.tensor_scalar_mul(
                        out=out_sb[:, :HID // 2],
                        in0=out_psum[:, :HID // 2],
                        scalar1=rstd_m[:, r:r + 1])
                    nc.scalar.activation(
                        out=out_sb[:, HID // 2:], in_=out_psum[:, HID // 2:],
                        func=mybir.ActivationFunctionType.Copy,
                        scale=rstd_m[:, r:r + 1])
                    nc.scalar.dma_start(
                        out=out[r * CHUNK:(r + 1) * CHUNK, :], in_=out_sb)

    nc.compile()
    return nc


_PROGRAM = None


def _get_program():
    global _PROGRAM
    if _PROGRAM is None:
        _PROGRAM = _build_program()
    return _PROGRAM


def _prep_inputs(hidden_states, positions, Wqkv, q_ln_w, k_ln_w, Wg, g_norm_w,
                 Wo):
    x = np.asarray(hidden_states, dtype=np.float32)
    pos = np.asarray(positions).astype(np.float32)

    # x transposed+tiled: xt[n, p, ko, t] = x[n*128+t, ko*128+p]
    xt = np.ascontiguousarray(
        x.reshape(NCHUNK, CHUNK, HID // 128, 128).transpose(0, 3, 2, 1)
    ).astype(BF16)

    # rope tables, duplicated per head-half layout [c, h*64+f]
    inv_freq = (1.0 / (ROPE_BASE ** (np.arange(HALF, dtype=np.float32) / HALF))
                ).astype(np.float32)
    ang = pos[:, None] * inv_freq[None, :]              # [T, HALF]
    cos = np.cos(ang).astype(np.float32)
    sin = np.sin(ang).astype(np.float32)
    cosd = np.tile(cos.reshape(NCHUNK, CHUNK, 1, HALF), (1, 1, 2, 1))
    sind = np.tile(sin.reshape(NCHUNK, CHUNK, 1, HALF), (1, 1, 2, 1))
    cosd = cosd.reshape(NCHUNK, CHUNK, D).astype(BF16)
    sind = sind.reshape(NCHUNK, CHUNK, D).astype(BF16)

    maskt = np.triu(np.ones((CHUNK, CHUNK), dtype=np.float32)).astype(BF16)

    qln2 = (np.tile(np.asarray(q_ln_w, np.float32), HPC) * SCALE)
    kln2 = np.tile(np.asarray(k_ln_w, np.float32), HPC)
    qln_b = np.broadcast_to(qln2, (128, CPC)).copy()
    kln_b = np.broadcast_to(kln2, (128, CPC)).copy()

    Wqkv = np.asarray(Wqkv, np.float32)
    Wg_ = np.asarray(Wg, np.float32)
    Wo_ = np.asarray(Wo, np.float32)
    gn = np.asarray(g_norm_w, np.float32)

    # full Wo, [p, ko, n] layout (same for every core)
    wo_r = np.ascontiguousarray(
        Wo_.reshape(HID // 128, 128, HID).transpose(1, 0, 2)
    ).astype(BF16)

    in_maps = []
    for c in range(N_CORES):
        lo, hi = c * CPC, (c + 1) * CPC
        wc = np.concatenate(
            [Wqkv[:, lo:hi], Wqkv[:, INNER + lo:INNER + hi],
             Wqkv[:, 2 * INNER + lo:2 * INNER + hi], Wg_[:, lo:hi]], axis=1)
        w_r = np.ascontiguousarray(
            wc.reshape(HID // 128, 128, 4 * CPC).transpose(1, 0, 2)
        ).astype(BF16)
        gnw_b = np.broadcast_to(gn[lo:hi], (128, CPC)).copy()
        in_maps.append({
            "xt": xt, "w": w_r, "wo": wo_r, "cosd": cosd, "sind": sind,
            "qln": qln_b, "kln": kln_b, "gnw": gnw_b, "maskt": maskt,
        })
    return in_maps


def run(trace=False, **inputs):
    nc = _get_program()
    in_maps = _prep_inputs(**inputs)
    res = run_bass_kernel_spmd(nc, in_maps, list(range(N_CORES)), trace=trace)
    # core c's row i = (2q+lh)*128 + c128  <->  token q*2048 + c*256
    #                                               + lh*128 + c128
    stacked = np.stack([res.results[c]["out"] for c in range(N_CORES)])
    full = (stacked.reshape(N_CORES, 4, 2, CHUNK, HID)
            .transpose(1, 0, 2, 3, 4).reshape(T, HID))
    return full, res


def kernel(**inputs):
    full, _ = run(trace=False, **inputs)
    return full


# revision 17
# speedup vs baseline: 1.0349x; 1.0349x over previous
"""Bass/Trainium2 kernel for nn_BailingMoELinearAttention.

Tensor-parallel over heads across 8 NeuronCores: each core owns 2 of the 16
heads (columns of Wqkv/Wg, rows of Wo). Per-core pipeline, fused per 128-token
chunk:
  qkvg projection (bf16 matmul) -> silu(q,k,v) / sigmoid(gate) -> per-head
  RMSNorm -> RoPE (host-precomputed cos/sin tables) -> chunked causal linear
  attention (running k^T v state, one PSUM bank per head) -> pre-gated
  activations g2 = o * g_norm_w * sigmoid(gate) and partial sum-of-squares.
Cross-core: AllReduce of per-token sum-of-squares (the group norm spans all 16
heads), per-token rstd scaling, then an AllToAll that exchanges the bf16 gated
activations so each core holds all 2048 inner columns for its 1024-token row
block, and a local out-projection against the full Wo. Host concatenates the 8
row blocks.
"""

import os
import sys

if "/opt/trn_rl_repo" not in sys.path:
    sys.path.insert(0, "/opt/trn_rl_repo")

import numpy as np
import ml_dtypes

import concourse.bass as bass
import concourse.tile as tile
from concourse import bacc, mybir
from concourse.bass_utils import run_bass_kernel_spmd
from concourse.masks import make_identity
from concourse.tile import add_dep_helper

BF16 = ml_dtypes.bfloat16

# Problem shape (hardcoded per contract).
T = 8192
HID = 2048
H = 16
D = 128
INNER = H * D
CHUNK = 128
NCHUNK = T // CHUNK  # 64
EPS = 1e-5
SCALE = D ** -0.5
ROPE_BASE = 600000.0
HALF = D // 2

N_CORES = 8
HPC = H // N_CORES          # 2 heads per core
CPC = HPC * D               # 256 inner cols per core
ROWS_PC = T // N_CORES      # 1024 output rows per core
RCHUNK = ROWS_PC // CHUNK   # 8 row-chunks per core in phase D

FP32 = mybir.dt.float32
BF = mybir.dt.bfloat16


def _build_program():
    nc = bacc.Bacc(
        "TRN2",
        target_bir_lowering=False,
        debug=False,
        num_devices=N_CORES,
    )

    # ---- DRAM I/O ----
    xt = nc.dram_tensor("xt", [NCHUNK, 128, HID // 128, CHUNK], BF,
                        kind="ExternalInput").ap()
    w = nc.dram_tensor("w", [128, HID // 128, 4 * CPC], BF,
                       kind="ExternalInput").ap()
    wo = nc.dram_tensor("wo", [128, HID // 128, HID], BF,
                        kind="ExternalInput").ap()
    cosd = nc.dram_tensor("cosd", [NCHUNK, CHUNK, D], BF,
                          kind="ExternalInput").ap()
    sind = nc.dram_tensor("sind", [NCHUNK, CHUNK, D], BF,
                          kind="ExternalInput").ap()
    qln = nc.dram_tensor("qln", [128, CPC], FP32, kind="ExternalInput").ap()
    kln = nc.dram_tensor("kln", [128, CPC], FP32, kind="ExternalInput").ap()
    gnw = nc.dram_tensor("gnw", [128, CPC], FP32, kind="ExternalInput").ap()
    maskt = nc.dram_tensor("maskt", [CHUNK, CHUNK], BF,
                           kind="ExternalInput").ap()
    out = nc.dram_tensor("out", [ROWS_PC, HID], FP32,
                         kind="ExternalOutput").ap()
    DEBUG = bool(os.environ.get("KB_DEBUG"))
    if DEBUG:
        dbg_o = nc.dram_tensor("dbg_o", [NCHUNK, 128, CPC], FP32,
                               kind="ExternalOutput").ap()
        dbg_ss = nc.dram_tensor("dbg_ss", [128, NCHUNK], FP32,
                                kind="ExternalOutput").ap()

    KO = HID // 128  # 16 k-chunks for the projections

    with tile.TileContext(nc) as tc:
        with (
            tc.tile_pool(name="const", bufs=1) as const,
            tc.tile_pool(name="persist", bufs=1) as persist,
            tc.tile_pool(name="dram", bufs=1, space="DRAM") as dram,
        ):
            # internal DRAM (A2A/RS split into 4 token-quarters so the
            # first three collectives overlap the chunk loop)
            TQ = T // 4
            a2a_in = [dram.tile([TQ, CPC], BF, name=f"a2a_in{q}")
                      for q in range(4)]
            a2a_out = [dram.tile([TQ, CPC], BF, name=f"a2a_out{q}")
                       for q in range(4)]
            ss_part = [dram.tile([TQ], FP32, name=f"ss_part{q}")
                       for q in range(4)]
            ss_mine = [dram.tile([TQ // N_CORES], FP32, name=f"ss_mine{q}")
                       for q in range(4)]

            # prefetch chunk 0 inputs ahead of the weight loads
            pre_xt = const.tile([128, KO, CHUNK], BF)
            nc.sync.dma_start(out=pre_xt, in_=xt[0])
            pre_cos = const.tile([CHUNK, D], BF)
            nc.sync.dma_start(out=pre_cos, in_=cosd[0])
            pre_sin = const.tile([CHUNK, D], BF)
            nc.sync.dma_start(out=pre_sin, in_=sind[0])

            # constants in SBUF
            w_s = const.tile([128, KO, 4 * CPC], BF)
            for ko in range(KO):
                nc.scalar.dma_start(out=w_s[:, ko, :], in_=w[:, ko, :])
            wo_s = const.tile([128, KO, HID], BF)
            qln_s = const.tile([128, CPC], FP32)
            nc.sync.dma_start(out=qln_s, in_=qln)
            kln_s = const.tile([128, CPC], FP32)
            nc.sync.dma_start(out=kln_s, in_=kln)
            gnw_s = const.tile([128, CPC], FP32)
            nc.sync.dma_start(out=gnw_s, in_=gnw)
            maskt_s = const.tile([CHUNK, CHUNK], BF)
            nc.sync.dma_start(out=maskt_s, in_=maskt)
            ident = const.tile([128, 128], BF)
            make_identity(nc, ident)
            identf = const.tile([128, 128], FP32)
            make_identity(nc, identf)
            eps_t = const.tile([128, 1], FP32)
            nc.vector.memset(eps_t, EPS)

            # accumulators living across the whole chunk loop
            ssm = persist.tile([128, RCHUNK], FP32)
            rmsm = persist.tile([128, RCHUNK], FP32)
            rstd_m = persist.tile([128, RCHUNK], FP32)
            ss_all = persist.tile([128, NCHUNK], FP32)     # partial sumsq
            s_sb = persist.tile([128, HPC, D], BF)         # state for o_inter
            nc.vector.memset(s_sb, 0.0)
            aiv = [a2a_in[q][:].rearrange("(n p) c -> n p c", p=CHUNK)
                   for q in range(4)]

            # ---- phase A/B/C: projections + attention, per 128-token chunk ----
            with (
                tc.tile_pool(name="xt_p", bufs=3) as xt_p,
                tc.tile_pool(name="trig", bufs=2) as trig,
                tc.tile_pool(name="work", bufs=2) as work,
                tc.tile_pool(name="small", bufs=4) as small,
                tc.tile_pool(name="pq", bufs=2, space="PSUM") as pq,
                tc.tile_pool(name="ptr", bufs=1, space="PSUM") as ptr,
                tc.tile_pool(name="po", bufs=1, space="PSUM") as po,
                tc.tile_pool(name="ps", bufs=1, space="PSUM") as ps,
            ):
                # one PSUM bank per head: a start=True from another group
                # sharing the bank would clear this group's has_written bits
                # and break the running accumulation
                s_psum = [ps.tile([128, D], FP32, tag=f"s{h}",
                                  name=f"s_psum{h}")
                          for h in range(HPC)]

                for n in range(NCHUNK):
                    # --- projection: qkvg chunk = x_chunk @ [Wq|Wk|Wv|Wg] ---
                    if n == 0:
                        xt_s = pre_xt
                    else:
                        xt_s = xt_p.tile([128, KO, CHUNK], BF)
                        xt_dma = nc.sync.dma_start(out=xt_s, in_=xt[n])
                        if n == 40:
                            xt40_dma = xt_dma
                    psum_qkvg = pq.tile([128, 4 * CPC], FP32)
                    for ko in range(KO):
                        for sl in range(2):
                            nc.tensor.matmul(
                                psum_qkvg[:, sl * 512:(sl + 1) * 512],
                                lhsT=xt_s[:, ko, :],
                                rhs=w_s[:, ko, sl * 512:(sl + 1) * 512],
                                start=(ko == 0),
                                stop=(ko == KO - 1),
                            )

                    # --- silu(qkv), sigmoid(gate) * g_norm_w ---
                    qkv_s = work.tile([128, 3 * CPC], FP32, tag="qkv")
                    nc.scalar.activation(
                        out=qkv_s, in_=psum_qkvg[:, :3 * CPC],
                        func=mybir.ActivationFunctionType.Silu,
                    )
                    sig_t = work.tile([128, CPC], BF, tag="sig")
                    nc.scalar.activation(
                        out=sig_t, in_=psum_qkvg[:, 3 * CPC:],
                        func=mybir.ActivationFunctionType.Sigmoid,
                    )
                    sg_t = work.tile([128, CPC], BF, tag="sg")
                    nc.vector.tensor_mul(sg_t, sig_t, gnw_s)

                    # --- per-head RMSNorm of q and k (over D=128) ---
                    sq_s = work.tile([128, 2 * CPC], FP32, tag="sq")
                    nc.vector.tensor_mul(sq_s, qkv_s[:, :2 * CPC],
                                         qkv_s[:, :2 * CPC])
                    ssum = small.tile([128, 4], FP32, tag="ssum")
                    nc.vector.reduce_sum(
                        out=ssum,
                        in_=sq_s.rearrange("p (g d) -> p g d", d=D),
                        axis=mybir.AxisListType.X,
                    )
                    rms = small.tile([128, 4], FP32, tag="rms")
                    nc.scalar.activation(
                        out=rms, in_=ssum,
                        func=mybir.ActivationFunctionType.Sqrt,
                        bias=eps_t, scale=1.0 / D,
                    )
                    rstd = small.tile([128, 4], FP32, tag="rstd")
                    nc.vector.reciprocal(out=rstd, in_=rms)
                    for g in range(4):
                        nc.vector.tensor_scalar_mul(
                            out=qkv_s[:, g * D:(g + 1) * D],
                            in0=qkv_s[:, g * D:(g + 1) * D],
                            scalar1=rstd[:, g:g + 1],
                        )
                    # ln weights (SCALE folded into qln on host)
                    nc.vector.tensor_mul(qkv_s[:, :CPC], qkv_s[:, :CPC], qln_s)
                    nc.vector.tensor_mul(qkv_s[:, CPC:2 * CPC],
                                         qkv_s[:, CPC:2 * CPC], kln_s)

                    # --- rope on q and k ---
                    if n == 0:
                        cos_t, sin_t = pre_cos, pre_sin
                    else:
                        cos_t = trig.tile([CHUNK, D], BF, tag="cos")
                        nc.sync.dma_start(out=cos_t, in_=cosd[n])
                        sin_t = trig.tile([CHUNK, D], BF, tag="sin")
                        nc.sync.dma_start(out=sin_t, in_=sind[n])
                    cos_v = cos_t.rearrange("p (h f) -> p h f", f=HALF)
                    sin_v = sin_t.rearrange("p (h f) -> p h f", f=HALF)

                    q_ro = work.tile([128, HPC, D], BF, tag="q_ro")
                    k_ro = work.tile([128, HPC, D], BF, tag="k_ro")
                    v_s = work.tile([128, HPC, D], BF, tag="v_s")
                    nc.vector.tensor_copy(v_s.rearrange("p h d -> p (h d)"),
                                          qkv_s[:, 2 * CPC:])
                    for src_off, dst in ((0, q_ro), (CPC, k_ro)):
                        xv = qkv_s[:, src_off:src_off + CPC].rearrange(
                            "p (h d) -> p h d", d=D)
                        x1 = xv[:, :, :HALF]
                        x2 = xv[:, :, HALF:]
                        t1 = work.tile([128, HPC, HALF], FP32, tag="ropet1")
                        t2 = work.tile([128, HPC, HALF], FP32, tag="ropet2")
                        nc.vector.tensor_mul(t1, x1, cos_v)
                        nc.vector.tensor_mul(t2, x2, sin_v)
                        nc.vector.tensor_sub(dst[:, :, :HALF], t1, t2)
                        nc.vector.tensor_mul(t1, x2, cos_v)
                        nc.vector.tensor_mul(t2, x1, sin_v)
                        nc.vector.tensor_add(dst[:, :, HALF:], t1, t2)

                    # --- transposes: qT, kT (per head) via PE ---
                    qT = work.tile([128, HPC, D], BF, tag="qT")
                    kT = work.tile([128, HPC, D], BF, tag="kT")
                    for ro, tr in ((q_ro, qT), (k_ro, kT)):
                        for h in range(HPC):
                            tp = ptr.tile([128, 128], BF, tag="tr")
                            nc.tensor.transpose(tp, ro[:, h, :], ident)
                            nc.vector.tensor_copy(tr[:, h, :], tp)

                    # --- attention ---
                    psum_o = po.tile([128, CPC], FP32)
                    prev_last_mm = None
                    for h in range(HPC):
                        sc_p = ptr.tile([128, 128], FP32, tag="tr")
                        nc.tensor.matmul(sc_p, lhsT=kT[:, h, :],
                                         rhs=qT[:, h, :], start=True, stop=True)
                        sc_s = work.tile([CHUNK, CHUNK], BF, tag="sc")
                        nc.vector.tensor_mul(sc_s, sc_p, maskt_s)
                        mm_intra = nc.tensor.matmul(
                            psum_o[:, h * D:(h + 1) * D],
                            lhsT=sc_s, rhs=v_s[:, h, :],
                            start=True, stop=(n == 0))
                        if prev_last_mm is not None:
                            # psum_o's bank is shared by both heads' groups;
                            # this start=True clears the whole bank's
                            # has_written bits, so it must not land between
                            # the other head's accumulating matmuls
                            add_dep_helper(mm_intra.ins, prev_last_mm.ins,
                                           reason="shared o-bank group order")
                        last_mm = mm_intra
                        if n > 0:
                            last_mm = nc.tensor.matmul(
                                psum_o[:, h * D:(h + 1) * D],
                                lhsT=qT[:, h, :], rhs=s_sb[:, h, :],
                                start=False, stop=True)
                        prev_last_mm = last_mm
                        if n < NCHUNK - 1:
                            nc.tensor.matmul(s_psum[h],
                                             lhsT=k_ro[:, h, :],
                                             rhs=v_s[:, h, :],
                                             start=(n == 0),
                                             stop=(n == NCHUNK - 2))
                            nc.vector.tensor_copy(s_sb[:, h, :], s_psum[h])

                    # --- g2 = o * (gnw * sig) -> a2a_in (rstd applied later,
                    # it commutes with the out-projection row-wise) ---
                    g2_t = work.tile([128, CPC], BF, tag="g2")
                    nc.vector.tensor_mul(g2_t, psum_o, sg_t)
                    # store via gpsimd (SWDGE): keeps the sync HWDGE queue
                    # free of stores so xt prefetches are never head-of-line
                    # blocked behind not-yet-ready data
                    nc.scalar.dma_start(out=aiv[n // 16][n % 16], in_=g2_t)
                    # partial sumsq of o (f32, straight from psum)
                    osq = work.tile([128, CPC], FP32, tag="osq")
                    nc.scalar.activation(
                        out=osq, in_=psum_o,
                        func=mybir.ActivationFunctionType.Square,
                    )
                    nc.vector.reduce_sum(out=ss_all[:, n:n + 1], in_=osq,
                                         axis=mybir.AxisListType.X)
                    if DEBUG:
                        do_ = work.tile([128, CPC], FP32, tag="dbgo")
                        nc.vector.tensor_copy(do_, psum_o)
                        nc.scalar.dma_start(out=dbg_o[n], in_=do_)

                    if n % 16 == 15:
                        # quarter q complete: token-major ss slice, then
                        # A2A of the quarter's gated activations + RS of
                        # its sumsq -- the first three quarters' collectives
                        # overlap the remaining chunk loop
                        q = n // 16
                        ssT_p = ptr.tile([16, 128], FP32, tag="tr",
                                         name=f"ssT_p{q}")
                        nc.tensor.transpose(
                            ssT_p, ss_all[:, q * 16:(q + 1) * 16], identf)
                        ssT = small.tile([16, 128], FP32, tag="ssT",
                                         name=f"ssT{q}")
                        nc.vector.tensor_copy(ssT, ssT_p)
                        nc.scalar.dma_start(
                            out=ss_part[q][:].rearrange(
                                "(n c) -> n c", c=CHUNK),
                            in_=ssT)
                        nc.gpsimd.collective_compute(
                            "ReduceScatter",
                            mybir.AluOpType.add,
                            replica_groups=[list(range(N_CORES))],
                            ins=[ss_part[q][:].opt()],
                            outs=[ss_mine[q][:].opt()],
                        )
                        nc.gpsimd.collective_compute(
                            "AllToAll",
                            mybir.AluOpType.bypass,
                            replica_groups=[list(range(N_CORES))],
                            ins=[a2a_in[q][:].opt()],
                            outs=[a2a_out[q][:].opt()],
                        )
            if DEBUG:
                nc.scalar.dma_start(out=dbg_ss, in_=ss_all)


            def emit_rstd(q):
                nc.sync.dma_start(
                    out=ssm[:, q * 2:(q + 1) * 2],
                    in_=ss_mine[q][:].rearrange("(r c) -> c r", c=CHUNK))
                nc.scalar.activation(
                    out=rmsm[:, q * 2:(q + 1) * 2],
                    in_=ssm[:, q * 2:(q + 1) * 2],
                    func=mybir.ActivationFunctionType.Sqrt,
                    bias=eps_t, scale=1.0 / INNER,
                )
                nc.vector.reciprocal(
                    out=rstd_m[:, q * 2:(q + 1) * 2],
                    in_=rmsm[:, q * 2:(q + 1) * 2])

            for q in range(3):
                emit_rstd(q)
            wo_dma = nc.sync.dma_start(out=wo_s, in_=wo)
            add_dep_helper(wo_dma.ins, xt40_dma.ins, sync=False,
                           reason="keep wo load out of the startup window")

            # ---- phase D: local out-projection of my token rows ----
            # a2a_out[q][j, lt, f] = gated cols [j*CPC+f] of token
            # q*2048 + me*256 + lt;  rchunk r covers (q, lh) = (r//2, r%2)
            aov = [a2a_out[q][:].rearrange("(j t) c -> j t c", j=N_CORES)
                   for q in range(4)]
            with (
                tc.tile_pool(name="dwork", bufs=2) as dwork,
                tc.tile_pool(name="gl", bufs=16) as gl,
                tc.tile_pool(name="pout", bufs=2, space="PSUM") as pout,
            ):
                for r in range(RCHUNK):
                    q, lh = r // 2, r % 2
                    if r == 6:
                        emit_rstd(3)
                    out_psum = pout.tile([128, HID], FP32)
                    for ko in range(KO):
                        j, half = ko // 2, ko % 2
                        glt = gl.tile([128, CHUNK], BF, tag="glt")
                        nc.sync.dma_start(
                            out=glt,
                            in_=aov[q][j, lh * CHUNK:(lh + 1) * CHUNK,
                                       half * 128:(half + 1) * 128],
                            transpose=True,
                        )
                        for sl in range(HID // 512):
                            nc.tensor.matmul(
                                out_psum[:, sl * 512:(sl + 1) * 512],
                                lhsT=glt,
                                rhs=wo_s[:, ko, sl * 512:(sl + 1) * 512],
                                start=(ko == 0),
                                stop=(ko == KO - 1),
                            )
                    out_sb = dwork.tile([128, HID], FP32, tag="out_sb")
                    nc.vector.tensor_scalar_mul(
                        out=out_sb[:, :HID // 2],
                        in0=out_psum[:, :HID // 2],
                        scalar1=rstd_m[:, r:r + 1])
                    nc.scalar.activation(
                        out=out_sb[:, HID // 2:], in_=out_psum[:, HID // 2:],
                        func=mybir.ActivationFunctionType.Copy,
                        scale=rstd_m[:, r:r + 1])
                    nc.scalar.dma_start(
                        out=out[r * CHUNK:(r + 1) * CHUNK, :], in_=out_sb)

    nc.compile()
    return nc


_PROGRAM = None


def _get_program():
    global _PROGRAM
    if _PROGRAM is None:
        _PROGRAM = _build_program()
    return _PROGRAM


def _prep_inputs(hidden_states, positions, Wqkv, q_ln_w, k_ln_w, Wg, g_norm_w,
                 Wo):
    x = np.asarray(hidden_states, dtype=np.float32)
    pos = np.asarray(positions).astype(np.float32)

    # x transposed+tiled: xt[n, p, ko, t] = x[n*128+t, ko*128+p]
    xt = np.ascontiguousarray(
        x.reshape(NCHUNK, CHUNK, HID // 128, 128).transpose(0, 3, 2, 1)
    ).astype(BF16)

    # rope tables, duplicated per head-half layout [c, h*64+f]
    inv_freq = (1.0 / (ROPE_BASE ** (np.arange(HALF, dtype=np.float32) / HALF))
                ).astype(np.float32)
    ang = pos[:, None] * inv_freq[None, :]              # [T, HALF]
    cos = np.cos(ang).astype(np.float32)
    sin = np.sin(ang).astype(np.float32)
    cosd = np.tile(cos.reshape(NCHUNK, CHUNK, 1, HALF), (1, 1, 2, 1))
    sind = np.tile(sin.reshape(NCHUNK, CHUNK, 1, HALF), (1, 1, 2, 1))
    cosd = cosd.reshape(NCHUNK, CHUNK, D).astype(BF16)
    sind = sind.reshape(NCHUNK, CHUNK, D).astype(BF16)

    maskt = np.triu(np.ones((CHUNK, CHUNK), dtype=np.float32)).astype(BF16)

    qln2 = (np.tile(np.asarray(q_ln_w, np.float32), HPC) * SCALE)
    kln2 = np.tile(np.asarray(k_ln_w, np.float32), HPC)
    qln_b = np.broadcast_to(qln2, (128, CPC)).copy()
    kln_b = np.broadcast_to(kln2, (128, CPC)).copy()

    Wqkv = np.asarray(Wqkv, np.float32)
    Wg_ = np.asarray(Wg, np.float32)
    Wo_ = np.asarray(Wo, np.float32)
    gn = np.asarray(g_norm_w, np.float32)

    # full Wo, [p, ko, n] layout (same for every core)
    wo_r = np.ascontiguousarray(
        Wo_.reshape(HID // 128, 128, HID).transpose(1, 0, 2)
    ).astype(BF16)

    in_maps = []
    for c in range(N_CORES):
        lo, hi = c * CPC, (c + 1) * CPC
        wc = np.concatenate(
            [Wqkv[:, lo:hi], Wqkv[:, INNER + lo:INNER + hi],
             Wqkv[:, 2 * INNER + lo:2 * INNER + hi], Wg_[:, lo:hi]], axis=1)
        w_r = np.ascontiguousarray(
            wc.reshape(HID // 128, 128, 4 * CPC).transpose(1, 0, 2)
        ).astype(BF16)
        gnw_b = np.broadcast_to(gn[lo:hi], (128, CPC)).copy()
        in_maps.append({
            "xt": xt, "w": w_r, "wo": wo_r, "cosd": cosd, "sind": sind,
            "qln": qln_b, "kln": kln_b, "gnw": gnw_b, "maskt": maskt,
        })
    return in_maps


def run(trace=False, **inputs):
    nc = _get_program()
    in_maps = _prep_inputs(**inputs)
    res = run_bass_kernel_spmd(nc, in_maps, list(range(N_CORES)), trace=trace)
    # core c's row i = (2q+lh)*128 + c128  <->  token q*2048 + c*256
    #                                               + lh*128 + c128
    stacked = np.stack([res.results[c]["out"] for c in range(N_CORES)])
    full = (stacked.reshape(N_CORES, 4, 2, CHUNK, HID)
            .transpose(1, 0, 2, 3, 4).reshape(T, HID))
    return full, res


def kernel(**inputs):
    full, _ = run(trace=False, **inputs)
    return full


# revision 18
# speedup vs baseline: 1.0361x; 1.0012x over previous
"""Bass/Trainium2 kernel for nn_BailingMoELinearAttention.

Tensor-parallel over heads across 8 NeuronCores: each core owns 2 of the 16
heads (columns of Wqkv/Wg, rows of Wo). Per-core pipeline, fused per 128-token
chunk:
  qkvg projection (bf16 matmul) -> silu(q,k,v) / sigmoid(gate) -> per-head
  RMSNorm -> RoPE (host-precomputed cos/sin tables) -> chunked causal linear
  attention (running k^T v state, one PSUM bank per head) -> pre-gated
  activations g2 = o * g_norm_w * sigmoid(gate) and partial sum-of-squares.
Cross-core: AllReduce of per-token sum-of-squares (the group norm spans all 16
heads), per-token rstd scaling, then an AllToAll that exchanges the bf16 gated
activations so each core holds all 2048 inner columns for its 1024-token row
block, and a local out-projection against the full Wo. Host concatenates the 8
row blocks.
"""

import os
import sys

if "/opt/trn_rl_repo" not in sys.path:
    sys.path.insert(0, "/opt/trn_rl_repo")

import numpy as np
import ml_dtypes

import concourse.bass as bass
import concourse.tile as tile
from concourse import bacc, mybir
from concourse.bass_utils import run_bass_kernel_spmd
from concourse.masks import make_identity
from concourse.tile import add_dep_helper

BF16 = ml_dtypes.bfloat16

# Problem shape (hardcoded per contract).
T = 8192
HID = 2048
H = 16
D = 128
INNER = H * D
CHUNK = 128
NCHUNK = T // CHUNK  # 64
EPS = 1e-5
SCALE = D ** -0.5
ROPE_BASE = 600000.0
HALF = D // 2

N_CORES = 8
HPC = H // N_CORES          # 2 heads per core
CPC = HPC * D               # 256 inner cols per core
ROWS_PC = T // N_CORES      # 1024 output rows per core
RCHUNK = ROWS_PC // CHUNK   # 8 row-chunks per core in phase D

FP32 = mybir.dt.float32
BF = mybir.dt.bfloat16


def _build_program():
    nc = bacc.Bacc(
        "TRN2",
        target_bir_lowering=False,
        debug=False,
        num_devices=N_CORES,
    )

    # ---- DRAM I/O ----
    xt = nc.dram_tensor("xt", [NCHUNK, 128, HID // 128, CHUNK], BF,
                        kind="ExternalInput").ap()
    w = nc.dram_tensor("w", [128, HID // 128, 4 * CPC], BF,
                       kind="ExternalInput").ap()
    wo = nc.dram_tensor("wo", [128, HID // 128, HID], BF,
                        kind="ExternalInput").ap()
    cosd = nc.dram_tensor("cosd", [NCHUNK, CHUNK, D], BF,
                          kind="ExternalInput").ap()
    sind = nc.dram_tensor("sind", [NCHUNK, CHUNK, D], BF,
                          kind="ExternalInput").ap()
    qln = nc.dram_tensor("qln", [128, CPC], FP32, kind="ExternalInput").ap()
    kln = nc.dram_tensor("kln", [128, CPC], FP32, kind="ExternalInput").ap()
    gnw = nc.dram_tensor("gnw", [128, CPC], FP32, kind="ExternalInput").ap()
    maskt = nc.dram_tensor("maskt", [CHUNK, CHUNK], BF,
                           kind="ExternalInput").ap()
    out = nc.dram_tensor("out", [ROWS_PC, HID], FP32,
                         kind="ExternalOutput").ap()
    DEBUG = bool(os.environ.get("KB_DEBUG"))
    if DEBUG:
        dbg_o = nc.dram_tensor("dbg_o", [NCHUNK, 128, CPC], FP32,
                               kind="ExternalOutput").ap()
        dbg_ss = nc.dram_tensor("dbg_ss", [128, NCHUNK], FP32,
                                kind="ExternalOutput").ap()

    KO = HID // 128  # 16 k-chunks for the projections

    with tile.TileContext(nc) as tc:
        with (
            tc.tile_pool(name="const", bufs=1) as const,
            tc.tile_pool(name="persist", bufs=1) as persist,
            tc.tile_pool(name="dram", bufs=1, space="DRAM") as dram,
        ):
            # internal DRAM (A2A/RS split into 4 token-quarters so the
            # first three collectives overlap the chunk loop)
            TQ = T // 4
            a2a_in = [dram.tile([TQ, CPC], BF, name=f"a2a_in{q}")
                      for q in range(4)]
            a2a_out = [dram.tile([TQ, CPC], BF, name=f"a2a_out{q}")
                       for q in range(4)]
            ss_part = [dram.tile([TQ], FP32, name=f"ss_part{q}")
                       for q in range(4)]
            ss_mine = [dram.tile([TQ // N_CORES], FP32, name=f"ss_mine{q}")
                       for q in range(4)]

            # prefetch chunk 0 inputs ahead of the weight loads
            pre_xt = const.tile([128, KO, CHUNK], BF)
            nc.sync.dma_start(out=pre_xt, in_=xt[0])
            pre_cos = const.tile([CHUNK, D], BF)
            nc.sync.dma_start(out=pre_cos, in_=cosd[0])
            pre_sin = const.tile([CHUNK, D], BF)
            nc.sync.dma_start(out=pre_sin, in_=sind[0])

            # constants in SBUF
            w_s = const.tile([128, KO, 4 * CPC], BF)
            for ko in range(KO):
                nc.scalar.dma_start(out=w_s[:, ko, :], in_=w[:, ko, :])
            wo_s = const.tile([128, KO, HID], BF)
            qln_s = const.tile([128, CPC], FP32)
            nc.sync.dma_start(out=qln_s, in_=qln)
            kln_s = const.tile([128, CPC], FP32)
            nc.sync.dma_start(out=kln_s, in_=kln)
            gnw_s = const.tile([128, CPC], FP32)
            nc.sync.dma_start(out=gnw_s, in_=gnw)
            maskt_s = const.tile([CHUNK, CHUNK], BF)
            nc.sync.dma_start(out=maskt_s, in_=maskt)
            ident = const.tile([128, 128], BF)
            make_identity(nc, ident)
            identf = const.tile([128, 128], FP32)
            make_identity(nc, identf)
            eps_t = const.tile([128, 1], FP32)
            nc.vector.memset(eps_t, EPS)

            # accumulators living across the whole chunk loop
            ssm = persist.tile([128, RCHUNK], FP32)
            rmsm = persist.tile([128, RCHUNK], FP32)
            rstd_m = persist.tile([128, RCHUNK], FP32)
            ss_all = persist.tile([128, NCHUNK], FP32)     # partial sumsq
            s_sb = persist.tile([128, HPC, D], BF)         # state for o_inter
            nc.vector.memset(s_sb, 0.0)
            aiv = [a2a_in[q][:].rearrange("(n p) c -> n p c", p=CHUNK)
                   for q in range(4)]

            # ---- phase A/B/C: projections + attention, per 128-token chunk ----
            with (
                tc.tile_pool(name="xt_p", bufs=3) as xt_p,
                tc.tile_pool(name="trig", bufs=2) as trig,
                tc.tile_pool(name="work", bufs=2) as work,
                tc.tile_pool(name="small", bufs=4) as small,
                tc.tile_pool(name="pq", bufs=2, space="PSUM") as pq,
                tc.tile_pool(name="ptr", bufs=1, space="PSUM") as ptr,
                tc.tile_pool(name="po", bufs=1, space="PSUM") as po,
                tc.tile_pool(name="ps", bufs=1, space="PSUM") as ps,
            ):
                # one PSUM bank per head: a start=True from another group
                # sharing the bank would clear this group's has_written bits
                # and break the running accumulation
                s_psum = [ps.tile([128, D], FP32, tag=f"s{h}",
                                  name=f"s_psum{h}")
                          for h in range(HPC)]

                for n in range(NCHUNK):
                    # --- projection: qkvg chunk = x_chunk @ [Wq|Wk|Wv|Wg] ---
                    if n == 0:
                        xt_s = pre_xt
                    else:
                        xt_s = xt_p.tile([128, KO, CHUNK], BF)
                        xt_dma = nc.sync.dma_start(out=xt_s, in_=xt[n])
                        if n == 40:
                            xt40_dma = xt_dma
                    psum_qkvg = pq.tile([128, 4 * CPC], FP32)
                    for ko in range(KO):
                        for sl in range(2):
                            nc.tensor.matmul(
                                psum_qkvg[:, sl * 512:(sl + 1) * 512],
                                lhsT=xt_s[:, ko, :],
                                rhs=w_s[:, ko, sl * 512:(sl + 1) * 512],
                                start=(ko == 0),
                                stop=(ko == KO - 1),
                            )

                    # --- silu(qkv), sigmoid(gate) * g_norm_w ---
                    qkv_s = work.tile([128, 3 * CPC], FP32, tag="qkv")
                    nc.scalar.activation(
                        out=qkv_s, in_=psum_qkvg[:, :3 * CPC],
                        func=mybir.ActivationFunctionType.Silu,
                    )
                    sig_t = work.tile([128, CPC], BF, tag="sig")
                    nc.scalar.activation(
                        out=sig_t, in_=psum_qkvg[:, 3 * CPC:],
                        func=mybir.ActivationFunctionType.Sigmoid,
                    )
                    sg_t = work.tile([128, CPC], BF, tag="sg")
                    nc.vector.tensor_mul(sg_t, sig_t, gnw_s)

                    # --- per-head RMSNorm of q and k (over D=128) ---
                    sq_s = work.tile([128, 2 * CPC], FP32, tag="sq")
                    nc.vector.tensor_mul(sq_s, qkv_s[:, :2 * CPC],
                                         qkv_s[:, :2 * CPC])
                    ssum = small.tile([128, 4], FP32, tag="ssum")
                    nc.vector.reduce_sum(
                        out=ssum,
                        in_=sq_s.rearrange("p (g d) -> p g d", d=D),
                        axis=mybir.AxisListType.X,
                    )
                    rms = small.tile([128, 4], FP32, tag="rms")
                    nc.scalar.activation(
                        out=rms, in_=ssum,
                        func=mybir.ActivationFunctionType.Sqrt,
                        bias=eps_t, scale=1.0 / D,
                    )
                    rstd = small.tile([128, 4], FP32, tag="rstd")
                    nc.vector.reciprocal(out=rstd, in_=rms)
                    for g in range(4):
                        nc.vector.tensor_scalar_mul(
                            out=qkv_s[:, g * D:(g + 1) * D],
                            in0=qkv_s[:, g * D:(g + 1) * D],
                            scalar1=rstd[:, g:g + 1],
                        )
                    # ln weights (SCALE folded into qln on host)
                    nc.vector.tensor_mul(qkv_s[:, :CPC], qkv_s[:, :CPC], qln_s)
                    nc.vector.tensor_mul(qkv_s[:, CPC:2 * CPC],
                                         qkv_s[:, CPC:2 * CPC], kln_s)

                    # --- rope on q and k ---
                    if n == 0:
                        cos_t, sin_t = pre_cos, pre_sin
                    else:
                        cos_t = trig.tile([CHUNK, D], BF, tag="cos")
                        nc.sync.dma_start(out=cos_t, in_=cosd[n])
                        sin_t = trig.tile([CHUNK, D], BF, tag="sin")
                        nc.sync.dma_start(out=sin_t, in_=sind[n])
                    cos_v = cos_t.rearrange("p (h f) -> p h f", f=HALF)
                    sin_v = sin_t.rearrange("p (h f) -> p h f", f=HALF)

                    q_ro = work.tile([128, HPC, D], BF, tag="q_ro")
                    k_ro = work.tile([128, HPC, D], BF, tag="k_ro")
                    v_s = work.tile([128, HPC, D], BF, tag="v_s")
                    nc.vector.tensor_copy(v_s.rearrange("p h d -> p (h d)"),
                                          qkv_s[:, 2 * CPC:])
                    for src_off, dst in ((0, q_ro), (CPC, k_ro)):
                        xv = qkv_s[:, src_off:src_off + CPC].rearrange(
                            "p (h d) -> p h d", d=D)
                        x1 = xv[:, :, :HALF]
                        x2 = xv[:, :, HALF:]
                        t1 = work.tile([128, HPC, HALF], FP32, tag="ropet1")
                        t2 = work.tile([128, HPC, HALF], FP32, tag="ropet2")
                        nc.vector.tensor_mul(t1, x1, cos_v)
                        nc.vector.tensor_mul(t2, x2, sin_v)
                        nc.vector.tensor_sub(dst[:, :, :HALF], t1, t2)
                        nc.vector.tensor_mul(t1, x2, cos_v)
                        nc.vector.tensor_mul(t2, x1, sin_v)
                        nc.vector.tensor_add(dst[:, :, HALF:], t1, t2)

                    # --- transposes: qT, kT (per head) via PE ---
                    qT = work.tile([128, HPC, D], BF, tag="qT")
                    kT = work.tile([128, HPC, D], BF, tag="kT")
                    for ro, tr in ((q_ro, qT), (k_ro, kT)):
                        for h in range(HPC):
                            tp = ptr.tile([128, 128], BF, tag="tr")
                            nc.tensor.transpose(tp, ro[:, h, :], ident)
                            nc.vector.tensor_copy(tr[:, h, :], tp)

                    # --- attention ---
                    psum_o = po.tile([128, CPC], FP32)
                    prev_last_mm = None
                    for h in range(HPC):
                        sc_p = ptr.tile([128, 128], FP32, tag="tr")
                        nc.tensor.matmul(sc_p, lhsT=kT[:, h, :],
                                         rhs=qT[:, h, :], start=True, stop=True)
                        sc_s = work.tile([CHUNK, CHUNK], BF, tag="sc")
                        nc.vector.tensor_mul(sc_s, sc_p, maskt_s)
                        mm_intra = nc.tensor.matmul(
                            psum_o[:, h * D:(h + 1) * D],
                            lhsT=sc_s, rhs=v_s[:, h, :],
                            start=True, stop=(n == 0))
                        if prev_last_mm is not None:
                            # psum_o's bank is shared by both heads' groups;
                            # this start=True clears the whole bank's
                            # has_written bits, so it must not land between
                            # the other head's accumulating matmuls
                            add_dep_helper(mm_intra.ins, prev_last_mm.ins,
                                           reason="shared o-bank group order")
                        last_mm = mm_intra
                        if n > 0:
                            last_mm = nc.tensor.matmul(
                                psum_o[:, h * D:(h + 1) * D],
                                lhsT=qT[:, h, :], rhs=s_sb[:, h, :],
                                start=False, stop=True)
                        prev_last_mm = last_mm
                        if n < NCHUNK - 1:
                            nc.tensor.matmul(s_psum[h],
                                             lhsT=k_ro[:, h, :],
                                             rhs=v_s[:, h, :],
                                             start=(n == 0),
                                             stop=(n == NCHUNK - 2))
                            nc.vector.tensor_copy(s_sb[:, h, :], s_psum[h])

                    # --- g2 = o * (gnw * sig) -> a2a_in (rstd applied later,
                    # it commutes with the out-projection row-wise) ---
                    g2_t = work.tile([128, CPC], BF, tag="g2")
                    nc.vector.tensor_mul(g2_t, psum_o, sg_t)
                    # store via gpsimd (SWDGE): keeps the sync HWDGE queue
                    # free of stores so xt prefetches are never head-of-line
                    # blocked behind not-yet-ready data
                    nc.scalar.dma_start(out=aiv[n // 16][n % 16], in_=g2_t)
                    # partial sumsq of o (f32, straight from psum)
                    osq = work.tile([128, CPC], FP32, tag="osq")
                    nc.scalar.activation(
                        out=osq, in_=psum_o,
                        func=mybir.ActivationFunctionType.Square,
                    )
                    nc.vector.reduce_sum(out=ss_all[:, n:n + 1], in_=osq,
                                         axis=mybir.AxisListType.X)
                    if DEBUG:
                        do_ = work.tile([128, CPC], FP32, tag="dbgo")
                        nc.vector.tensor_copy(do_, psum_o)
                        nc.scalar.dma_start(out=dbg_o[n], in_=do_)

                    if n % 16 == 15:
                        # quarter q complete: token-major ss slice, then
                        # A2A of the quarter's gated activations + RS of
                        # its sumsq -- the first three quarters' collectives
                        # overlap the remaining chunk loop
                        q = n // 16
                        ssT_p = ptr.tile([16, 128], FP32, tag="tr",
                                         name=f"ssT_p{q}")
                        nc.tensor.transpose(
                            ssT_p, ss_all[:, q * 16:(q + 1) * 16], identf)
                        ssT = small.tile([16, 128], FP32, tag="ssT",
                                         name=f"ssT{q}")
                        nc.vector.tensor_copy(ssT, ssT_p)
                        nc.scalar.dma_start(
                            out=ss_part[q][:].rearrange(
                                "(n c) -> n c", c=CHUNK),
                            in_=ssT)
                        nc.gpsimd.collective_compute(
                            "ReduceScatter",
                            mybir.AluOpType.add,
                            replica_groups=[list(range(N_CORES))],
                            ins=[ss_part[q][:].opt()],
                            outs=[ss_mine[q][:].opt()],
                        )
                        nc.gpsimd.collective_compute(
                            "AllToAll",
                            mybir.AluOpType.bypass,
                            replica_groups=[list(range(N_CORES))],
                            ins=[a2a_in[q][:].opt()],
                            outs=[a2a_out[q][:].opt()],
                        )
                        nc.gpsimd.dma_start(
                            out=ssm[:, q * 2:(q + 1) * 2],
                            in_=ss_mine[q][:].rearrange(
                                "(r c) -> c r", c=CHUNK))
            if DEBUG:
                nc.scalar.dma_start(out=dbg_ss, in_=ss_all)


            def emit_rstd(q):
                nc.scalar.activation(
                    out=rmsm[:, q * 2:(q + 1) * 2],
                    in_=ssm[:, q * 2:(q + 1) * 2],
                    func=mybir.ActivationFunctionType.Sqrt,
                    bias=eps_t, scale=1.0 / INNER,
                )
                nc.vector.reciprocal(
                    out=rstd_m[:, q * 2:(q + 1) * 2],
                    in_=rmsm[:, q * 2:(q + 1) * 2])

            for q in range(3):
                emit_rstd(q)
            wo_dma = nc.sync.dma_start(out=wo_s, in_=wo)
            add_dep_helper(wo_dma.ins, xt40_dma.ins, sync=False,
                           reason="keep wo load out of the startup window")

            # ---- phase D: local out-projection of my token rows ----
            # a2a_out[q][j, lt, f] = gated cols [j*CPC+f] of token
            # q*2048 + me*256 + lt;  rchunk r covers (q, lh) = (r//2, r%2)
            aov = [a2a_out[q][:].rearrange("(j t) c -> j t c", j=N_CORES)
                   for q in range(4)]
            with (
                tc.tile_pool(name="dwork", bufs=2) as dwork,
                tc.tile_pool(name="gl", bufs=16) as gl,
                tc.tile_pool(name="pout", bufs=2, space="PSUM") as pout,
            ):
                for r in range(RCHUNK):
                    q, lh = r // 2, r % 2
                    if r == 6:
                        emit_rstd(3)
                    out_psum = pout.tile([128, HID], FP32)
                    for ko in range(KO):
                        j, half = ko // 2, ko % 2
                        glt = gl.tile([128, CHUNK], BF, tag="glt")
                        nc.sync.dma_start(
                            out=glt,
                            in_=aov[q][j, lh * CHUNK:(lh + 1) * CHUNK,
                                       half * 128:(half + 1) * 128],
                            transpose=True,
                        )
                        for sl in range(HID // 512):
                            nc.tensor.matmul(
                                out_psum[:, sl * 512:(sl + 1) * 512],
                                lhsT=glt,
                                rhs=wo_s[:, ko, sl * 512:(sl + 1) * 512],
                                start=(ko == 0),
                                stop=(ko == KO - 1),
                            )
                    out_sb = dwork.tile([128, HID], FP32, tag="out_sb")
                    nc.vector.tensor_scalar_mul(
                        out=out_sb[:, :HID // 2],
                        in0=out_psum[:, :HID // 2],
                        scalar1=rstd_m[:, r:r + 1])
                    nc.scalar.activation(
                        out=out_sb[:, HID // 2:], in_=out_psum[:, HID // 2:],
                        func=mybir.ActivationFunctionType.Copy,
                        scale=rstd_m[:, r:r + 1])
                    nc.scalar.dma_start(
                        out=out[r * CHUNK:(r + 1) * CHUNK, :], in_=out_sb)

    nc.compile()
    return nc


_PROGRAM = None


def _get_program():
    global _PROGRAM
    if _PROGRAM is None:
        _PROGRAM = _build_program()
    return _PROGRAM


def _prep_inputs(hidden_states, positions, Wqkv, q_ln_w, k_ln_w, Wg, g_norm_w,
                 Wo):
    x = np.asarray(hidden_states, dtype=np.float32)
    pos = np.asarray(positions).astype(np.float32)

    # x transposed+tiled: xt[n, p, ko, t] = x[n*128+t, ko*128+p]
    xt = np.ascontiguousarray(
        x.reshape(NCHUNK, CHUNK, HID // 128, 128).transpose(0, 3, 2, 1)
    ).astype(BF16)

    # rope tables, duplicated per head-half layout [c, h*64+f]
    inv_freq = (1.0 / (ROPE_BASE ** (np.arange(HALF, dtype=np.float32) / HALF))
                ).astype(np.float32)
    ang = pos[:, None] * inv_freq[None, :]              # [T, HALF]
    cos = np.cos(ang).astype(np.float32)
    sin = np.sin(ang).astype(np.float32)
    cosd = np.tile(cos.reshape(NCHUNK, CHUNK, 1, HALF), (1, 1, 2, 1))
    sind = np.tile(sin.reshape(NCHUNK, CHUNK, 1, HALF), (1, 1, 2, 1))
    cosd = cosd.reshape(NCHUNK, CHUNK, D).astype(BF16)
    sind = sind.reshape(NCHUNK, CHUNK, D).astype(BF16)

    maskt = np.triu(np.ones((CHUNK, CHUNK), dtype=np.float32)).astype(BF16)

    qln2 = (np.tile(np.asarray(q_ln_w, np.float32), HPC) * SCALE)
    kln2 = np.tile(np.asarray(k_ln_w, np.float32), HPC)
    qln_b = np.broadcast_to(qln2, (128, CPC)).copy()
    kln_b = np.broadcast_to(kln2, (128, CPC)).copy()

    Wqkv = np.asarray(Wqkv, np.float32)
    Wg_ = np.asarray(Wg, np.float32)
    Wo_ = np.asarray(Wo, np.float32)
    gn = np.asarray(g_norm_w, np.float32)

    # full Wo, [p, ko, n] layout (same for every core)
    wo_r = np.ascontiguousarray(
        Wo_.reshape(HID // 128, 128, HID).transpose(1, 0, 2)
    ).astype(BF16)

    in_maps = []
    for c in range(N_CORES):
        lo, hi = c * CPC, (c + 1) * CPC
        wc = np.concatenate(
            [Wqkv[:, lo:hi], Wqkv[:, INNER + lo:INNER + hi],
             Wqkv[:, 2 * INNER + lo:2 * INNER + hi], Wg_[:, lo:hi]], axis=1)
        w_r = np.ascontiguousarray(
            wc.reshape(HID // 128, 128, 4 * CPC).transpose(1, 0, 2)
        ).astype(BF16)
        gnw_b = np.broadcast_to(gn[lo:hi], (128, CPC)).copy()
        in_maps.append({
            "xt": xt, "w": w_r, "wo": wo_r, "cosd": cosd, "sind": sind,
            "qln": qln_b, "kln": kln_b, "gnw": gnw_b, "maskt": maskt,
        })
    return in_maps


def run(trace=False, **inputs):
    nc = _get_program()
    in_maps = _prep_inputs(**inputs)
    res = run_bass_kernel_spmd(nc, in_maps, list(range(N_CORES)), trace=trace)
    # core c's row i = (2q+lh)*128 + c128  <->  token q*2048 + c*256
    #                                               + lh*128 + c128
    stacked = np.stack([res.results[c]["out"] for c in range(N_CORES)])
    full = (stacked.reshape(N_CORES, 4, 2, CHUNK, HID)
            .transpose(1, 0, 2, 3, 4).reshape(T, HID))
    return full, res


def kernel(**inputs):
    full, _ = run(trace=False, **inputs)
    return full
